# revision 11
# baseline (speedup 1.0000x reference)
"""AdaptiveWaveletNet Trainium2 kernel (8 NeuronCores, batch-data-parallel).

Self-contained: hardcodes shapes from the problem spec.
  x: (32, 128, 4096) f32;  LEVELS=3, NF=8, KSZ=8, K=4.

Strategy (v2):
  - Batch B=32 sharded across 8 cores (4 samples/core); params replicated.
  - Channels C=128 on SBUF partitions. All four per-core samples live
    side-by-side in each SBUF tile (P, BL, n) and are processed in lockstep,
    so elementwise ops batch across samples (one instruction, 3-d APs with
    broadcast LN params) and the per-branch scalar statistics chains run
    once per level on (P, BL) tiles instead of per sample.
  - Convs on TensorE as diagonal-matrix matmuls in fp16 (full-rate),
    accumulated in fp32 PSUM; reflect-pad boundary columns via tiny
    reversed-AP matmuls; outputs split on PSUM bank boundaries.
  - ScalarE does Gelu/Square/Copy only (all in one ACT table set - no table
    reloads) with free per-partition accumulation sums.
  - LayerNorm cross-partition stats via an all-ones matmul (sum+broadcast in
    one PE op). rsqrt computed on VectorE with a Quake-seed + 2 Newton
    iterations, batched (P, BL) - avoids ScalarE Sqrt table switches.
  - Signals stored fp16 (conv operands need 16-bit; DVE gets 2x/4x modes);
    stats/accumulations in fp32. Outputs written fp16, upcast on host.
  - Kurtosis/energy/regu partial sums go to a small f32 stats output;
    final tiny reductions on host in float64.
"""
import sys

sys.path.insert(0, "/opt/trn_rl_repo")

import numpy as np

import concourse.bass as bass
import concourse.bacc as bacc
import concourse.tile as tile
from concourse import mybir
from concourse.bass_utils import run_bass_kernel_spmd

P = 128          # channels == SBUF partitions
B = 32           # global batch
NCORES = 8
BL = B // NCORES # samples per core
L = 4096
LEVELS = 3
K = 4            # lifting dw-conv taps
KSZ = 8          # wavelet filter taps
NF = 8
CH = 1024        # psum chunk columns
EPS_LN = 1e-5
EPS_IN = 1e-5

f32 = mybir.dt.float32
f16 = mybir.dt.float16
i32 = mybir.dt.int32
ALU = mybir.AluOpType
ACTF = mybir.ActivationFunctionType
AX = mybir.AxisListType

# diag matrix slot layout inside the "diag" input (P, NDIAG, P)
def U_DIAG(l, k): return 4 * l + k
def P_DIAG(l, k): return 12 + 4 * l + k
def LO_DIAG(k): return 24 + k
def HI_DIAG(k): return 32 + k
NDIAG = 40

# ln param plane layout inside "lnp" (P, 4, CHAIN): u_ln_w, u_ln_b, p_ln_w, p_ln_b
ULW, ULB, PLW, PLB = 0, 1, 2, 3

RSQRT_MAGIC = 0x5F3759DF


def level_sizes(Lfull=L):
    return [Lfull >> (l + 1) for l in range(LEVELS)]


def chain_offsets(Lfull=L):
    ns = level_sizes(Lfull)
    off, out = 0, []
    for n in ns:
        out.append(off)
        off += n
    return out, off


def stat_layout():
    """Columns of the per-sample (P, BL, NSTAT) stats output tile."""
    cols, i = {}, 0
    for l in range(LEVELS):
        for key in ("k_s1", "k_s2", "k_s4", "e_lo", "e_hi", "a_d"):
            cols[(key, l)] = i
            i += 1
    cols["apx_s1"] = i
    i += 1
    return cols, i


STAT_COLS, NSTAT = stat_layout()


def make_sig(base2d, par, stride):
    """Logical per-sample signal v[i] = base2d[:, par + stride*i]."""
    pstep, pcount = base2d.ap[0]
    estep = base2d.ap[1][0]

    def view(i0, cnt, rev=False):
        stepmul = -stride if rev else stride
        return bass.AP(
            tensor=base2d.tensor,
            offset=base2d.offset + (par + stride * i0) * estep,
            ap=[[pstep, pcount], [estep * stepmul, cnt]],
        )

    return view


def bcast_free(ap2d, count):
    """(P, n) -> (P, count, n) with a step-0 broadcast middle dim."""
    pstep, pcount = ap2d.ap[0]
    estep, ecnt = ap2d.ap[1]
    return bass.AP(tensor=ap2d.tensor, offset=ap2d.offset,
                   ap=[[pstep, pcount], [0, count], [estep, ecnt]])


def strided3(base2d, par, stride, nsamp, samp_step, cnt):
    """(P, nsamp, cnt) view: v[b, i] = base2d[:, par + b*samp_step + stride*i]."""
    pstep, pcount = base2d.ap[0]
    estep = base2d.ap[1][0]
    return bass.AP(tensor=base2d.tensor,
                   offset=base2d.offset + par * estep,
                   ap=[[pstep, pcount], [samp_step * estep, nsamp],
                       [stride * estep, cnt]])


def build_program(BL_=BL, Lfull=L):
    ns = level_sizes(Lfull)
    choff, chain = chain_offsets(Lfull)
    nc = bacc.Bacc(None, target_bir_lowering=False)

    x_ext = nc.declare_dram_parameter("x", [BL_, P, Lfull], f16, isOutput=False)
    lnp_ext = nc.declare_dram_parameter("lnp", [P, 4, chain], f16, isOutput=False)
    bias_ext = nc.declare_dram_parameter("bias", [P, 6], f32, isOutput=False)
    diag_ext = nc.declare_dram_parameter("diag", [P, NDIAG, P], f16, isOutput=False)

    dn_ext = [
        nc.declare_dram_parameter(f"d{l}n", [BL_, P, ns[l]], f16, isOutput=True)
        for l in range(LEVELS)
    ]
    apx_ext = nc.declare_dram_parameter("apxn", [BL_, P, ns[-1]], f16, isOutput=True)
    st_ext = nc.declare_dram_parameter("stats", [BL_, P, NSTAT], f32, isOutput=True)

    from contextlib import ExitStack

    with tile.TileContext(nc) as tc, ExitStack() as ctx:
        singles = ctx.enter_context(tc.tile_pool(name="singles", bufs=1))
        bigs = ctx.enter_context(tc.tile_pool(name="bigs", bufs=1))
        gp = ctx.enter_context(tc.tile_pool(name="gp", bufs=2))
        tiny = ctx.enter_context(tc.tile_pool(name="tiny", bufs=4))
        stp = ctx.enter_context(tc.tile_pool(name="stp", bufs=1))
        pup = ctx.enter_context(tc.tile_pool(name="pup", bufs=3, space="PSUM"))
        psp = ctx.enter_context(tc.tile_pool(name="psp", bufs=2, space="PSUM"))

        # --- resident params ---
        lnp_s = singles.tile([P, 4, chain], f16)
        nc.sync.dma_start(out=lnp_s[:], in_=lnp_ext[:])
        bias_s = singles.tile([P, 6], f32)
        nc.sync.dma_start(out=bias_s[:], in_=bias_ext[:])
        diag_s = singles.tile([P, NDIAG, P], f16)
        nc.sync.dma_start(out=diag_s[:], in_=diag_ext[:])
        ones_s = singles.tile([P, P], f32)
        nc.vector.memset(ones_s[:], 1.0)
        magic_s = singles.tile([P, BL_], i32)
        nc.vector.memset(magic_s[:], RSQRT_MAGIC)

        # --- whole-quad tiles, reused across levels ---
        x_t = bigs.tile([P, BL_, Lfull], f16)
        c1 = bigs.tile([P, BL_, chain], f16)
        c2 = bigs.tile([P, BL_, chain], f16)
        lo_t = bigs.tile([P, BL_, ns[0]], f16)
        hi_t = bigs.tile([P, BL_, ns[0]], f16)
        d_t = bigs.tile([P, BL_, ns[0]], f16)
        st = stp.tile([P, BL_, NSTAT], f32)

        def ln_ap(plane, lvl):
            return lnp_s[:, plane, choff[lvl]:choff[lvl] + ns[lvl]]

        def emit_conv(pu, t0, cn, view, n, tap_slots, pl):
            """Accumulate out cols [t0, t0+cn) of the reflect-pad conv into pu."""
            Kt = len(tap_slots)
            mms = []
            for k in [pl] + [k for k in range(Kt) if k != pl]:
                a = max(t0, pl - k)
                bnd = min(t0 + cn - 1, n - 1 + pl - k)
                if a <= bnd:
                    mms.append((k, a, bnd - a + 1, a + k - pl, 1))
                if k < pl and t0 == 0:
                    mms.append((k, 0, pl - k, pl - k, -1))
                if k > pl and t0 + cn >= n:
                    mms.append((k, n - (k - pl), k - pl, n - 2, -1))
            pieces = []
            for (k, ta, cnt, v0, vstep) in mms:
                while cnt > 0:
                    rel = ta - t0
                    take = min(cnt, 512 - (rel % 512))
                    pieces.append((k, ta, take, v0, vstep))
                    ta += take
                    v0 += vstep * take
                    cnt -= take
            banks_started = set()
            for i, (k, ta, cnt, v0, vstep) in enumerate(pieces):
                rhs = view(v0, cnt, vstep < 0)
                bank = (ta - t0) // 512
                nc.tensor.matmul(
                    pu[:, ta - t0:ta - t0 + cnt], diag_s[:, tap_slots[k], :], rhs,
                    start=(bank not in banks_started),
                    stop=(i == len(pieces) - 1), skip_group_check=True,
                )
                banks_started.add(bank)

        def emit_rsqrt(al, a, nb):
            """al (P, nb) f32 <- 1/sqrt(a); a (P, nb) f32, positive.

            Quake seed on the DVE int path + 2 Newton iterations."""
            ai = a.bitcast(i32)
            sh = tiny.tile([P, BL_], i32, tag="nr_sh")
            nc.vector.tensor_scalar(sh[:, :nb], ai, 1, None,
                                    op0=ALU.arith_shift_right)
            yi = tiny.tile([P, BL_], i32, tag="nr_yi")
            nc.vector.tensor_tensor(yi[:, :nb], magic_s[:, :nb], sh[:, :nb],
                                    op=ALU.subtract)
            y = yi.bitcast(f32)
            ah = tiny.tile([P, BL_], f32, tag="nr_ah")
            nc.vector.tensor_scalar(ah[:, :nb], a, -0.5, None, op0=ALU.mult)
            q = tiny.tile([P, BL_], f32, tag="nr_q")
            s = tiny.tile([P, BL_], f32, tag="nr_s")
            for _ in range(2):
                nc.vector.tensor_tensor(q[:, :nb], y[:, :nb], y[:, :nb], op=ALU.mult)
                nc.vector.tensor_tensor(q[:, :nb], q[:, :nb], ah[:, :nb], op=ALU.mult)
                nc.vector.tensor_scalar(s[:, :nb], q[:, :nb], 1.5, None, op0=ALU.add)
                nc.vector.tensor_tensor(y[:, :nb], y[:, :nb], s[:, :nb], op=ALU.mult)
            nc.vector.tensor_copy(al[:, :nb], y[:, :nb])

        def quad_branch(dst3, base3, sign, conv_views, n, tap_slots,
                        bias_ap, lnw_ap, lnb_ap, accums=None):
            """dst3 (P, BL, n) <- base3 (+/-) LN2d(gelu(conv+bias))*lnw + lnb.

            conv_views: per-sample conv-input view fns. base3: (P, BL, n) AP.
            accums: optional per-sample (P,1) accum_out targets for sum(dst).
            Returns the g scratch tile (P, BL, ns[0]) f16.
            """
            nc.vector.tensor_tensor(dst3, base3, bcast_free(lnb_ap, BL_),
                                    op=(ALU.add if sign > 0 else ALU.subtract))
            g = gp.tile([P, BL_, ns[0]], f16, tag="g")
            nch = (n + CH - 1) // CH
            sti = tiny.tile([P, 2 * BL_ * nch], f32, tag="sti")
            for b in range(BL_):
                for ci in range(nch):
                    cn = min(CH, n - ci * CH)
                    pu = pup.tile([P, CH], f32, tag="pu")
                    emit_conv(pu[:, :cn], ci * CH, cn, conv_views[b], n,
                              tap_slots, K // 2)
                    c0 = ci * CH
                    base = 2 * nch * b
                    nc.scalar.activation(g[:, b, c0:c0 + cn], pu[:, :cn],
                                         ACTF.Gelu, bias=bias_ap, scale=1.0,
                                         accum_out=sti[:, base + ci:base + ci + 1])
                    nc.scalar.activation(pu[:, :cn], g[:, b, c0:c0 + cn],
                                         ACTF.Square,
                                         accum_out=sti[:, base + nch + ci:
                                                       base + nch + ci + 1])
            # cross-partition sum + broadcast, all samples at once
            ncols = 2 * BL_ * nch
            ps = psp.tile([P, 16], f32, tag="ps")
            nc.tensor.matmul(ps[:, :ncols], ones_s[:], sti[:, :ncols],
                             start=True, stop=True)
            ms = tiny.tile([P, 2 * BL_], f32, tag="ms")  # [b][{s1,s2}]
            if nch > 1:
                psv = ps[:, :ncols].rearrange("p (bj nch) -> p bj nch", nch=nch)
                nc.vector.tensor_reduce(ms[:], psv, axis=AX.X, op=ALU.add)
                nc.vector.tensor_scalar(ms[:], ms[:], 1.0 / (P * n), None,
                                        op0=ALU.mult)
            else:
                nc.vector.tensor_scalar(ms[:], ps[:, :ncols], 1.0 / (P * n), None,
                                        op0=ALU.mult)
            msv = ms[:].rearrange("p (b two) -> p b two", two=2)
            var = tiny.tile([P, BL_], f32, tag="var")
            nc.vector.tensor_tensor(var[:], msv[:, :, 0], msv[:, :, 0], op=ALU.mult)
            nc.vector.tensor_tensor(var[:], msv[:, :, 1], var[:], op=ALU.subtract)
            nc.vector.tensor_scalar(var[:], var[:], EPS_LN, None, op0=ALU.add)
            al = tiny.tile([P, BL_], f32, tag="al")
            emit_rsqrt(al, var[:], BL_)
            # t = (g - mu_b) * al_b  (two-scalar TS per sample, in place on g)
            for b in range(BL_):
                nc.vector.tensor_scalar(g[:, b, :n], g[:, b, :n],
                                        msv[:, b, 0:1], al[:, b:b + 1],
                                        op0=ALU.subtract, op1=ALU.mult)
            # sw = t * lnw (batched, broadcast lnw); dst = dst +/- sw
            nc.vector.tensor_tensor(g[:, :, :n], g[:, :, :n],
                                    bcast_free(lnw_ap, BL_), op=ALU.mult)
            if accums is None:
                nc.vector.tensor_tensor(dst3, dst3, g[:, :, :n],
                                        op=(ALU.add if sign > 0 else ALU.subtract))
            else:
                for b in range(BL_):
                    nc.vector.scalar_tensor_tensor(
                        dst3[:, b], g[:, b, :n], float(sign), dst3[:, b],
                        op0=ALU.mult, op1=ALU.add, accum_out=accums[b])
            return g

        def emit_mf(dst, views, n, tap_slots, abs_col3):
            """dst (P, BL, n) <- folded multi-filter conv; abs_col3 <- sum|dst|."""
            nch = (n + CH - 1) // CH
            for b in range(BL_):
                for ci in range(nch):
                    cn = min(CH, n - ci * CH)
                    pu = pup.tile([P, CH], f32, tag="pu")
                    emit_conv(pu[:, :cn], ci * CH, cn, views[b], n, tap_slots,
                              KSZ // 2 - 1)
                    nc.scalar.copy(dst[:, b, ci * CH:ci * CH + cn], pu[:, :cn])
            nc.vector.tensor_reduce(abs_col3, dst[:, :, :n], axis=AX.X, op=ALU.add,
                                    apply_absolute_value=True)

        def quad_inorm(t3, n, s1_cols, out3=None, eps=EPS_IN):
            """out3 <- per-(partition,sample) instance norm of t3 (P, BL, n).

            s1_cols: (P, BL) AP holding per-sample sums of t3."""
            q = gp.tile([P, BL_, ns[0]], f16, tag="g")
            s2 = tiny.tile([P, BL_], f32, tag="in_s2")
            for b in range(BL_):
                nc.scalar.activation(q[:, b, :n], t3[:, b, :n], ACTF.Square,
                                     accum_out=s2[:, b:b + 1])
            mu = tiny.tile([P, BL_], f32, tag="in_mu")
            nc.vector.tensor_scalar(mu[:], s1_cols, 1.0 / n, None, op0=ALU.mult)
            var = tiny.tile([P, BL_], f32, tag="in_var")
            nc.vector.tensor_scalar(var[:], s2[:], 1.0 / n, None, op0=ALU.mult)
            t = tiny.tile([P, BL_], f32, tag="in_t")
            nc.vector.tensor_tensor(t[:], mu[:], mu[:], op=ALU.mult)
            nc.vector.tensor_tensor(var[:], var[:], t[:], op=ALU.subtract)
            nc.vector.tensor_scalar(var[:], var[:], float(eps), None, op0=ALU.add)
            al = tiny.tile([P, BL_], f32, tag="in_al")
            emit_rsqrt(al, var[:], BL_)
            dst = t3 if out3 is None else out3
            for b in range(BL_):
                nc.vector.tensor_scalar(dst[:, b, :n], t3[:, b, :n],
                                        mu[:, b:b + 1], al[:, b:b + 1],
                                        op0=ALU.subtract, op1=ALU.mult)

        # --- load x (dram (BL,P,L) -> sbuf (P,BL,L)) ---
        nc.sync.dma_start(out=x_t[:], in_=x_ext[:].transpose([1, 0, 2]))

        def sample_slices(src, lvl):
            if lvl == 0:
                return [x_t[:, b, :] for b in range(BL_)]
            o = choff[lvl - 1]
            return [src[:, b, o:o + ns[lvl - 1]] for b in range(BL_)]

        def interleave3(curs, parity, n):
            """(P, BL, n) AP of curs[b][parity::2] (same layout each sample)."""
            base = curs[0]
            samp_step = curs[1].offset - curs[0].offset if BL_ > 1 else 0
            return strided3(base, parity, 2, BL_, samp_step, n)

        # ================= pass 1: kurtosis features =================
        for lvl in range(LEVELS):
            n = ns[lvl]
            curs = sample_slices(c1, lvl)
            odd_views = [make_sig(c, 1, 2) for c in curs]
            even3 = interleave3(curs, 0, n)
            odd3 = interleave3(curs, 1, n)
            cdst = c1[:, :, choff[lvl]:choff[lvl] + n]
            quad_branch(cdst, even3, +1, odd_views, n,
                        [U_DIAG(lvl, k) for k in range(K)],
                        bias_s[:, lvl:lvl + 1], ln_ap(ULW, lvl), ln_ap(ULB, lvl))
            cviews = [make_sig(c1[:, b, choff[lvl]:choff[lvl] + n], 0, 1)
                      for b in range(BL_)]
            s1c = STAT_COLS[("k_s1", lvl)]
            d_accs = [st[:, b, s1c:s1c + 1] for b in range(BL_)]
            g = quad_branch(d_t[:, :, :n], odd3, -1, cviews, n,
                            [P_DIAG(lvl, k) for k in range(K)],
                            bias_s[:, 3 + lvl:4 + lvl], ln_ap(PLW, lvl),
                            ln_ap(PLB, lvl), accums=d_accs)
            # kurtosis partials: s2 = sum d^2, s4 = sum (d-mu)^4 per partition
            mu = tiny.tile([P, BL_], f32, tag="k_mu")
            nc.vector.tensor_scalar(mu[:], st[:, :, s1c], 1.0 / n, None,
                                    op0=ALU.mult)
            m2n = tiny.tile([P, BL_], f32, tag="k_m2n")
            nc.vector.tensor_scalar(m2n[:], mu[:], -2.0, None, op0=ALU.mult)
            msq = tiny.tile([P, BL_], f32, tag="k_msq")
            nc.vector.tensor_tensor(msq[:], mu[:], mu[:], op=ALU.mult)
            s2c, s4c = STAT_COLS[("k_s2", lvl)], STAT_COLS[("k_s4", lvl)]
            for b in range(BL_):
                nc.scalar.activation(g[:, b, :n], d_t[:, b, :n], ACTF.Square,
                                     accum_out=st[:, b, s2c:s2c + 1])
                nc.vector.scalar_tensor_tensor(g[:, b, :n], d_t[:, b, :n],
                                               m2n[:, b:b + 1], g[:, b, :n],
                                               op0=ALU.mult, op1=ALU.add)
                nc.scalar.activation(g[:, b, :n], g[:, b, :n], ACTF.Square,
                                     bias=msq[:, b:b + 1], scale=1.0,
                                     accum_out=st[:, b, s4c:s4c + 1])

        # ================= pass 2: wavelet + lifting =================
        for lvl in range(LEVELS):
            n = ns[lvl]
            curs = sample_slices(c2, lvl)
            even_views = [make_sig(c, 0, 2) for c in curs]
            odd_views = [make_sig(c, 1, 2) for c in curs]
            elo = STAT_COLS[("e_lo", lvl)]
            ehi = STAT_COLS[("e_hi", lvl)]
            emit_mf(lo_t, even_views, n, [LO_DIAG(k) for k in range(KSZ)],
                    st[:, :, elo:elo + 1])
            emit_mf(hi_t, odd_views, n, [HI_DIAG(k) for k in range(KSZ)],
                    st[:, :, ehi:ehi + 1])
            cdst = c2[:, :, choff[lvl]:choff[lvl] + n]
            apx_accs = None
            if lvl == LEVELS - 1:
                ac = STAT_COLS["apx_s1"]
                apx_accs = [st[:, b, ac:ac + 1] for b in range(BL_)]
            quad_branch(cdst, lo_t[:, :, :n], +1,
                        [make_sig(hi_t[:, b, :n], 0, 1) for b in range(BL_)],
                        n, [U_DIAG(lvl, k) for k in range(K)],
                        bias_s[:, lvl:lvl + 1], ln_ap(ULW, lvl), ln_ap(ULB, lvl),
                        accums=apx_accs)
            ds1 = tiny.tile([P, BL_], f32, tag="p2_ds1")
            d_accs = [ds1[:, b:b + 1] for b in range(BL_)]
            quad_branch(d_t[:, :, :n], hi_t[:, :, :n], -1,
                        [make_sig(c2[:, b, choff[lvl]:choff[lvl] + n], 0, 1)
                         for b in range(BL_)],
                        n, [P_DIAG(lvl, k) for k in range(K)],
                        bias_s[:, 3 + lvl:4 + lvl], ln_ap(PLW, lvl),
                        ln_ap(PLB, lvl), accums=d_accs)
            adc = STAT_COLS[("a_d", lvl)]
            nc.vector.tensor_reduce(st[:, :, adc:adc + 1], d_t[:, :, :n],
                                    axis=AX.X, op=ALU.add,
                                    apply_absolute_value=True)
            quad_inorm(d_t[:, :, :n], n, ds1[:])
            nc.sync.dma_start(out=dn_ext[lvl][:].transpose([1, 0, 2]),
                              in_=d_t[:, :, :n])

        # approx: instance norm of final c2 level -> lo_t staging (free now)
        ac = STAT_COLS["apx_s1"]
        apx3 = c2[:, :, choff[-1]:choff[-1] + ns[-1]]
        quad_inorm(apx3, ns[-1], st[:, :, ac], out3=lo_t[:, :, :ns[-1]])
        nc.sync.dma_start(out=apx_ext[:].transpose([1, 0, 2]),
                          in_=lo_t[:, :, :ns[-1]])
        nc.sync.dma_start(out=st_ext[:].transpose([1, 0, 2]), in_=st[:])

    nc.finalize()
    return nc


# ---------------------------------------------------------------------------
# host side
# ---------------------------------------------------------------------------

def host_pack_params(lo_filters, hi_filters, filter_weights,
                     p_w, p_b, p_ln_w, p_ln_b, u_w, u_b, u_ln_w, u_ln_b,
                     Lfull=L):
    ns = level_sizes(Lfull)
    choff, chain = chain_offsets(Lfull)

    fw = np.asarray(filter_weights, np.float64)
    e = np.exp(fw - fw.max())
    w = e / e.sum()
    lo_taps = (w[:, None] * np.asarray(lo_filters, np.float64)[:, 0, :]).sum(0)
    hi_taps = (w[:, None] * np.asarray(hi_filters, np.float64)[:, 0, :]).sum(0)

    lnp = np.zeros((P, 4, chain), np.float32)
    for lvl in range(LEVELS):
        sl = slice(choff[lvl], choff[lvl] + ns[lvl])
        lnp[:, ULW, sl] = np.asarray(u_ln_w[lvl], np.float32)
        lnp[:, ULB, sl] = np.asarray(u_ln_b[lvl], np.float32)
        lnp[:, PLW, sl] = np.asarray(p_ln_w[lvl], np.float32)
        lnp[:, PLB, sl] = np.asarray(p_ln_b[lvl], np.float32)

    bias = np.zeros((P, 6), np.float32)
    for lvl in range(LEVELS):
        bias[:, lvl] = np.asarray(u_b[lvl], np.float32)
        bias[:, 3 + lvl] = np.asarray(p_b[lvl], np.float32)

    diag = np.zeros((P, NDIAG, P), np.float32)
    idx = np.arange(P)
    for lvl in range(LEVELS):
        for k in range(K):
            diag[idx, U_DIAG(lvl, k), idx] = np.asarray(u_w[lvl], np.float32)[:, 0, k]
            diag[idx, P_DIAG(lvl, k), idx] = np.asarray(p_w[lvl], np.float32)[:, 0, k]
    for k in range(KSZ):
        diag[idx, LO_DIAG(k), idx] = np.float32(lo_taps[k])
        diag[idx, HI_DIAG(k), idx] = np.float32(hi_taps[k])

    return lnp.astype(np.float16), bias, diag.astype(np.float16)


def host_epilogue(results, x_mean, lo_filters, hi_filters, Lfull=L, BL_=BL):
    """Combine per-core outputs into the full pytree."""
    ns = level_sizes(Lfull)
    approx_n = np.concatenate([r["apxn"].astype(np.float32) for r in results],
                              axis=0)
    details_n = tuple(
        np.concatenate([r[f"d{l}n"].astype(np.float32) for r in results], axis=0)
        for l in range(LEVELS)
    )
    stats = np.concatenate([r["stats"] for r in results], axis=0).astype(np.float64)
    Btot = stats.shape[0]

    def col(key, l=None):
        c = STAT_COLS[key if l is None else (key, l)]
        return stats[:, :, c]  # (Btot, P)

    trans = np.zeros((Btot, LEVELS))
    for l in range(LEVELS):
        n = ns[l]
        s1, s2, s4 = col("k_s1", l), col("k_s2", l), col("k_s4", l)
        mu = s1 / n
        var = np.maximum(s2 / n - mu * mu, 0.0)
        sig = np.maximum(np.sqrt(var), 1e-8)
        trans[:, l] = (s4 / (n * sig ** 4)).mean(axis=1) - 3.0

    energy = np.zeros((Btot, 2 * LEVELS))
    for l in range(LEVELS):
        energy[:, 2 * l] = col("e_lo", l).sum(axis=1) / (P * ns[l])
        energy[:, 2 * l + 1] = col("e_hi", l).sum(axis=1) / (P * ns[l])

    regu = 0.0
    for l in range(LEVELS):
        regu += 0.1 * col("a_d", l).sum() / (Btot * P * ns[l])
    apx_mean = col("apx_s1").sum() / (Btot * P * ns[-1])
    regu += 0.05 * abs(apx_mean - x_mean)
    lo2 = np.asarray(lo_filters, np.float64)[:, 0, :]
    hi2 = np.asarray(hi_filters, np.float64)[:, 0, :]
    regu += 0.01 * np.abs((lo2 * hi2).sum(axis=1)).sum()
    regu += 0.01 * (np.abs(np.linalg.norm(lo2, axis=1) - 1.0)
                    + np.abs(np.linalg.norm(hi2, axis=1) - 1.0)).sum()

    return (approx_n, details_n, np.float32(regu),
            energy.astype(np.float32), trans.astype(np.float32))


_PROGRAM_CACHE = {}


def _get_program():
    if "nc" not in _PROGRAM_CACHE:
        _PROGRAM_CACHE["nc"] = build_program()
    return _PROGRAM_CACHE["nc"]


def kernel(x, lo_filters, hi_filters, filter_weights,
           p_w, p_b, p_ln_w, p_ln_b, u_w, u_b, u_ln_w, u_ln_b):
    x = np.ascontiguousarray(np.asarray(x, np.float32))
    lnp, bias, diag = host_pack_params(
        lo_filters, hi_filters, filter_weights,
        p_w, p_b, p_ln_w, p_ln_b, u_w, u_b, u_ln_w, u_ln_b)

    nc = _get_program()
    xb = x.astype(np.float16)
    in_maps = []
    for c in range(NCORES):
        shard = np.ascontiguousarray(xb[c * BL:(c + 1) * BL])
        in_maps.append({"x": shard, "lnp": lnp, "bias": bias, "diag": diag})
    res = run_bass_kernel_spmd(nc, in_maps, core_ids=list(range(NCORES)))
    x_mean = float(x.mean(dtype=np.float64))
    return host_epilogue(res.results, x_mean, lo_filters, hi_filters)
